# revision 11
# baseline (speedup 1.0000x reference)
"""CTC-style loss (nn_CTCFormal) on 8 Trainium2 NeuronCores.

Pure data parallel over batch N=4096 -> 512 samples/core, laid out as
[P=128 partitions, G=4 groups] with the whole alpha recurrence on the
Vector engine (a GpSimd group-offload was tried and measured slower: Pool
TT ops have ~230 ns fixed cost vs DVE's ~50 ns marginal per group).

Formulation: the alpha recurrence is rescaled by the per-step blank
probability.  With a~[t,s] = alpha[t,s] / prod_{tau<=t} y_blank(tau):

  even s (blanks):  a~[t,s] = a~[t-1,s] + a~[t-1,s-1]           (no multiply)
  odd  s (labels):  a~[t,s] = (a~[t-1,s] + a~[t-1,s-1]
                               + a~[t-1,s-2] * z[j]) * r[t,j]
  loss = -( log(a~[T-1,S-1] + a~[T-1,S-2]) + sum_t x_blank[t] )

where r = exp(x_lab - x_blank) and z is the static CTC skip mask (0 when
lab[j]==lab[j-1]).  The blank product becomes a log-space reduce_sum of
raw blank logits, so the inner loop is 4 DVE ops/step and only the final
multiply reads ACT-produced data (one cross-engine wait per step).

The alpha DP is banded: at step t only states s in [max(0,2t-66),
min(62,2t+1)] are live (states outside cannot lie on any path from
(0,{0,1}) to (T-1,{S-1,S-2})), so ops slice just the live band (~53% of
the elements, exact).

a~ reaches ~1e22; the ACT Ln table is only accurate on ~[1e-15, 1e15], so
the final log runs with scale=2^-32 inside the activation and the 32*ln2
correction is folded into the last tensor_scalar.

Host prep replicates the reference's (buggy) target padding, gathers the
per-sample label-class logit rows (index-only data movement; this
environment's SWDGE gather ucode faults), subtracts the blank row, and
ships a bf16 [P, T, G, L] time-major tensor so each step's r slice is
contiguous.  Chunked DMAs + ACT exp (small chunks first, to lead the
early narrow-band steps) overlap the recurrence.  The host sums the
8x512 partials (the all-reduce of the scalar loss sum).
"""

import numpy as np

T, N, C = 64, 4096, 128
L = 31           # labels per sample
L2 = 32          # label dim padded even so fp32 copies hit the DVE 2x mode
S = 2 * L + 1    # 63 padded states
NCORES = 8
NLOC = N // NCORES          # 512 samples per core
G = NLOC // 128             # 4 groups of 128 samples (partition dim)
P = 128
CHUNKS = [2, 2, 2, 2, 4, 4, 4, 4, 8, 8, 8, 8, 8]   # T chunking for DMA/exp pipeline

_BASS_CACHE = {}


def _band(t):
    """Live CTC band [lo, hi] (inclusive states) at step t; lo forced even."""
    lo = max(0, 2 * t - 66)
    hi = min(S - 1, 2 * t + 1)
    return lo, hi


def _build_bass():
    if "nc" in _BASS_CACHE:
        return _BASS_CACHE["nc"]

    import concourse.bacc as bacc
    import concourse.mybir as mybir
    from concourse.tile import TileContext

    f32 = mybir.dt.float32
    bf16 = mybir.dt.bfloat16
    AF = mybir.ActivationFunctionType
    Alu = mybir.AluOpType

    nc = bacc.Bacc(trn_type="TRN2")
    xd_d = nc.declare_dram_parameter("xd", [P, T, G, L2], bf16, isOutput=False)
    xdm_d = nc.declare_dram_parameter("xdm", [P, T, G, L2], bf16, isOutput=False)
    blkl_d = nc.declare_dram_parameter("blkl", [P, G, T], f32, isOutput=False)
    loss_d = nc.declare_dram_parameter("loss", [P, G], f32, isOutput=True)

    with TileContext(nc) as tc:
        with tc.tile_pool(name="main", bufs=1) as pool:
            blkl = pool.tile([P, G, T], f32)
            nc.sync.dma_start(out=blkl[:], in_=blkl_d[:])
            # alpha states (cols 0,1 zero; state s in col s+2)
            a = pool.tile([P, G, S + 2], f32)
            b = pool.tile([P, G, S + 2], f32)
            vv = pool.tile([P, G, L], f32)
            nc.vector.memset(a[:], 0.0)
            nc.vector.memset(b[:], 0.0)
            nc.vector.memset(a[:, :, 2], 1.0)

            xd_s = pool.tile([P, T, G, L2], bf16)
            xdm_s = pool.tile([P, T, G, L2], bf16)
            r = pool.tile([P, T, G, L2], f32)
            r2 = pool.tile([P, T, G, L2], f32)
            # DVE-owned copies: every in-loop read then has only same-engine
            # waits (one HW wait slot per instruction; a second wait costs an
            # event-semaphore instruction per step), and the copy work soaks
            # up the step-boundary write-ack stalls
            rc = pool.tile([P, T, G, L2], f32)
            rc2 = pool.tile([P, T, G, L2], f32)
            off = 0
            for ci, ch in enumerate(CHUNKS):
                sl = slice(off, off + ch)
                off += ch
                nc.sync.dma_start(out=xd_s[:, sl], in_=xd_d[:, sl])
                nc.scalar.activation(out=r[:, sl], in_=xd_s[:, sl], func=AF.Exp)
                nc.sync.dma_start(out=xdm_s[:, sl], in_=xdm_d[:, sl])
                nc.scalar.activation(out=r2[:, sl], in_=xdm_s[:, sl], func=AF.Exp)
                if ci == 0:
                    # a~0[s=1] = r[t=0, j=0]; emitted here so ACT runs it
                    # before the later chunks' exps (ACT executes in order)
                    nc.scalar.copy(out=a[:, :, 3], in_=r[:, 0, :, 0])

            for c0 in (slice(0, 4), slice(4, 8)):
                nc.vector.tensor_copy(out=rc[:, c0], in_=r[:, c0])
                nc.vector.tensor_copy(out=rc2[:, c0], in_=r2[:, c0])

            cur, nxt = a, b
            next_copy = 8
            for t in range(1, T):
                if t % 4 == 2 and next_copy < T:
                    cs = slice(next_copy, next_copy + 4)
                    nc.vector.tensor_copy(out=rc[:, cs], in_=r[:, cs])
                    nc.vector.tensor_copy(out=rc2[:, cs], in_=r2[:, cs])
                    next_copy += 4
                lo, hi = _band(t)
                clo, chi = lo + 2, hi + 2
                ho = hi if hi % 2 == 1 else hi - 1   # top odd state
                jlo, jhi = lo // 2, (ho - 1) // 2    # inclusive label idx range
                # nxt[s] = cur[s] + cur[s-1] over the band (both parities)
                nc.vector.tensor_add(
                    out=nxt[:, :, clo : chi + 1],
                    in0=cur[:, :, clo : chi + 1],
                    in1=cur[:, :, clo - 1 : chi],
                )
                # vv[j] = a~[t-1, 2j-1] * r2[t,j]   (cur col 2j+1)
                nc.vector.tensor_mul(
                    out=vv[:, :, jlo : jhi + 1],
                    in0=cur[:, :, 2 * jlo + 1 : 2 * jhi + 2 : 2],
                    in1=rc2[:, t, :, jlo : jhi + 1],
                )
                # odd lanes: *= r, then += vv  (this order keeps only one
                # adjacent RAW pair per step; engines have no interlocks, so
                # a dependent op one slot behind pays the write-ack latency)
                nc.vector.tensor_mul(
                    out=nxt[:, :, 2 * jlo + 3 : 2 * jhi + 4 : 2],
                    in0=nxt[:, :, 2 * jlo + 3 : 2 * jhi + 4 : 2],
                    in1=rc[:, t, :, jlo : jhi + 1],
                )
                nc.vector.tensor_add(
                    out=nxt[:, :, 2 * jlo + 3 : 2 * jhi + 4 : 2],
                    in0=nxt[:, :, 2 * jlo + 3 : 2 * jhi + 4 : 2],
                    in1=vv[:, :, jlo : jhi + 1],
                )
                cur, nxt = nxt, cur

            # loss = -( log((a~[S-1]+a~[S-2]) * 2^-32) + 32*ln2 + sum_t x_blank )
            stot = pool.tile([P, G], f32)
            nc.vector.tensor_add(
                out=stot[:], in0=cur[:, :, S + 1], in1=cur[:, :, S]
            )
            lg = pool.tile([P, G], f32)
            nc.scalar.activation(
                out=lg[:], in_=stot[:], func=AF.Ln, scale=float(2.0**-32)
            )
            # blkl is uploaded with +32*ln2/T folded into every element, so
            # bsum already carries the Ln pre-scale correction
            bsum = pool.tile([P, G, 1], f32)
            nc.vector.reduce_sum(out=bsum[:], in_=blkl[:], axis=mybir.AxisListType.X)
            neg = pool.tile([P, G], f32)
            nc.vector.scalar_tensor_tensor(
                out=neg[:],
                in0=lg[:],
                scalar=-1.0,
                in1=bsum[:, :, 0],
                op0=Alu.mult,
                op1=Alu.subtract,
            )
            nc.sync.dma_start(out=loss_d[:], in_=neg[:])

    nc.finalize()
    _BASS_CACHE["nc"] = nc
    return nc


def host_prep(input, target, input_length, target_length):
    """Build the 8 per-core input maps."""
    import ml_dtypes

    inp = np.asarray(input, dtype=np.float32)
    target = np.asarray(target, dtype=np.int32)
    tl = np.asarray(target_length, dtype=np.int64)

    # reference's buggy padding: start_i = target_length[i-1] if i>0 else 0,
    # clamped like jax.lax.dynamic_slice
    starts = np.zeros(N, np.int64)
    starts[1:] = tl[: N - 1]
    starts = np.clip(starts, 0, len(target) - L)
    lab = target[starts[:, None] + np.arange(L)]  # [N, L] int32
    z = np.ones((N, L), np.float32)
    z[:, 1:] = (lab[:, 1:] != lab[:, :-1]).astype(np.float32)

    x_nct = inp.transpose(1, 2, 0)  # [N, C, T] view
    xs = np.take_along_axis(x_nct, lab[:, :, None].astype(np.int64), axis=1)
    blk = x_nct[:, 0, :]                       # [N, T]
    xd = xs - blk[:, None, :]                  # [N, L, T]
    xdm = np.where(z[:, :, None] == 0.0, np.float32(-1e30), xd)
    xd = np.pad(xd, ((0, 0), (0, 1), (0, 0)))      # [N, L2, T]
    xdm = np.pad(xdm, ((0, 0), (0, 1), (0, 0)))
    xd = np.ascontiguousarray(xd.transpose(0, 2, 1)).astype(ml_dtypes.bfloat16)
    xdm = np.ascontiguousarray(xdm.transpose(0, 2, 1)).astype(ml_dtypes.bfloat16)

    in_maps = []
    for core in range(NCORES):
        sl = slice(core * NLOC, (core + 1) * NLOC)
        xd_c = xd[sl].reshape(G, P, T, L2).transpose(1, 2, 0, 3)
        xdm_c = xdm[sl].reshape(G, P, T, L2).transpose(1, 2, 0, 3)
        blk_c = (blk[sl] + np.float32(32.0 * np.log(2.0) / T)).reshape(G, P, T).transpose(
            1, 0, 2
        )
        in_maps.append(
            {
                "xd": np.ascontiguousarray(xd_c),
                "xdm": np.ascontiguousarray(xdm_c),
                "blkl": np.ascontiguousarray(blk_c),
            }
        )
    return in_maps


def kernel(input, target, input_length, target_length):
    from concourse.bass_utils import run_bass_kernel_spmd

    nc = _build_bass()
    in_maps = host_prep(input, target, input_length, target_length)
    res = run_bass_kernel_spmd(nc, in_maps, list(range(NCORES)))
    total = 0.0
    for core in range(NCORES):
        total += float(np.asarray(res.results[core]["loss"], dtype=np.float64).sum())
    return np.float32(total)
